# revision 1
# baseline (speedup 1.0000x reference)
"""Trainium2 Bass kernel for nn_Attention (channel-attention, 8 NeuronCores).

Algorithm (algebraically identical to the reference):
  The attention contracts over the spatial axis n = 32*32*32 = 32768, and the
  attention matrices are tiny (64x64 per head).  Everything collapses around
  the per-batch Gram matrix G_b = x_b @ x_b^T (128x128):

    scores_bh = scale * Wq_h G_b Wk_h^T            (tiny)
    attn      = softmax(scores)                     (tiny)
    W_eff_b   = (1/n) * sum_h Wout_h attn_bh Wv_h   (64x128, tiny)
    y_b       = W_eff_b @ x_b + b_out               (the only other big matmul)

  Sharding: spatial n split across the 8 cores (4096 each).  Each core
  computes a partial G over its shard (the only big contraction), a 64KB
  bf16 AllReduce combines them, the tiny attention algebra is replicated on
  every core, and each core produces its own n-slice of y.  All matmuls run
  in bf16 with f32 accumulation (rounding errors average out over the huge
  contractions; measured end-to-end max rel err ~1e-4).

  x is shipped once, in [n, c] layout (needed by the Gram matmuls); the
  [c, n] layout needed by the final y matmul is produced on-chip with PE
  transposes scheduled under the AllReduce wait, which also keeps the PE
  HAM-warm through the collective.
"""

import numpy as np
import ml_dtypes

import concourse.bass as bass
import concourse.bacc as bacc
import concourse.mybir as mybir
import concourse.tile as tile
from concourse.tile import add_dep_helper
from concourse.bass_utils import run_bass_kernel_spmd

NCORES = 8
P = 128
N_TOT = 32 * 32 * 32          # 32768 spatial points
NSH = N_TOT // NCORES         # 4096 per core per batch
F = 2 * NSH                   # 8192 free columns (both batches side by side)
NCHUNK = 4                    # xn DMA chunks (pipelined with the G matmuls)
DUMMY_WARM_MMS = 38           # HAM warm-keeper fp32 matmuls under the AR wait
HEADS = 8
DH = 64
SCALE = DH ** -0.5
BF = mybir.dt.bfloat16
F32 = mybir.dt.float32
bf16 = ml_dtypes.bfloat16

_CACHED_NC = None


class _TrimmedTileContext(tile.TileContext):
    """TileContext minus the FINAL all-engine barrier of the exit sequence.

    The stock exit is drain -> barrier -> sem-clear -> barrier; the last
    barrier only makes every engine wait for the gpsimd sem-clear before
    halting, which matters for looped NEFFs but not a single-shot kernel:
    the clear still completes before its issuing engine halts, so a
    re-execution starts with zeroed semaphores either way.  Dropping it
    saves ~4us of measured EVSEM-butterfly tail.
    """

    def _drain_and_barrier(self, tick_clock, wait_clock):
        from concourse.vector_clock import ScopedClock

        drain_inst = self.nc.sync.drain()
        wait_clock.add_sem_waits(
            drain_inst.ins, ScopedClock({None: tick_clock.global_clock})
        )
        self.nc.all_engine_barrier()
        popped = self.nc._tile_sem_poison_stack.pop()
        assert popped is self._sem_poison
        self.nc.clear_and_free_semaphores(list(self.sems.allocated().values()))


def build_nc():
    nc = bacc.Bacc(
        "TRN2", target_bir_lowering=False, debug=False, num_devices=NCORES
    )

    xn_ext = nc.dram_tensor("xn", [P, F], BF, kind="ExternalInput")
    wq_ext = nc.dram_tensor("wqT", [P, 512], BF, kind="ExternalInput")
    wk_ext = nc.dram_tensor("wkT", [P, 512], BF, kind="ExternalInput")
    wv_ext = nc.dram_tensor("wv", [P, 512], BF, kind="ExternalInput")
    wo_ext = nc.dram_tensor("woT", [P, 256], BF, kind="ExternalInput")
    bo_ext = nc.dram_tensor("bout", [P, 1], F32, kind="ExternalInput")
    id_ext = nc.dram_tensor("ident", [P, P], BF, kind="ExternalInput")
    out_ext = nc.dram_tensor("out", [P, NSH], F32, kind="ExternalOutput")

    with _TrimmedTileContext(nc) as tc:
        with (
            tc.tile_pool(name="const", bufs=1) as const,
            tc.tile_pool(name="data", bufs=1) as data,
            tc.tile_pool(name="work", bufs=1) as work,
            tc.tile_pool(name="ypool", bufs=1) as ypool,
            tc.tile_pool(name="psg", bufs=2, space="PSUM") as psg,
            tc.tile_pool(name="psd", bufs=2, space="PSUM") as psd,
            tc.tile_pool(name="psy", bufs=2, space="PSUM") as psy,
            tc.tile_pool(name="dram", bufs=1, space="DRAM") as dram,
        ):
            # ---- ncfw warm-up: a tiny dependency-free AllReduce triggered
            # right after the preamble.  The real collective's doorbell is
            # otherwise noticed ~30-50us late (collective-firmware wakeup);
            # queueing this one first can absorb that latency.  Its own
            # completion is consumed by a throwaway DMA on an idle ring, and
            # since dummy-end (~trigger+stall+small mesh) always precedes the
            # real collective's own stall window, it can never delay it.
            warm_in = dram.tile([P, 2], F32, tag="warm_in")
            warm_out = dram.tile([P, 2], F32, tag="warm_out", addr_space="Shared")
            nc.gpsimd.collective_compute(
                "AllReduce",
                mybir.AluOpType.add,
                ins=[warm_in.opt()],
                outs=[warm_out.opt()],
                replica_groups=[[i] for i in range(NCORES)],
            )
            # ---- phase B: xn split across BOTH HWDGE rings; weights queue
            # behind the xn chunks (they are needed only much later) ----
            CH = F // NCHUNK  # 2048 columns (16 n-blocks) per chunk
            xn_tiles = []
            for c in range(NCHUNK):
                t = data.tile([P, CH], BF, tag=f"xn{c}")
                eng = nc.sync if c % 2 == 0 else nc.scalar
                eng.dma_start(t[:], xn_ext[:, c * CH : (c + 1) * CH])
                xn_tiles.append(t)

            wq = const.tile([P, 512], BF, tag="wq")
            nc.sync.dma_start(wq[:], wq_ext[:])
            wk = const.tile([P, 512], BF, tag="wk")
            nc.scalar.dma_start(wk[:], wk_ext[:])
            wv = const.tile([P, 512], BF, tag="wv")
            nc.sync.dma_start(wv[:], wv_ext[:])
            wo = const.tile([P, 256], BF, tag="wo")
            nc.scalar.dma_start(wo[:], wo_ext[:])
            bo = const.tile([P, 1], F32, tag="bo")
            nc.sync.dma_start(bo[:], bo_ext[:])
            ident = const.tile([P, P], BF, tag="ident")
            nc.scalar.dma_start(ident[:], id_ext[:])
            # warm-keeper source, zeroed early while the DVE is idle
            dummy_src = work.tile([P, 512], F32, tag="dummy")
            nc.vector.memset(dummy_src[:], 0.0)

            g_ps = [psg.tile([P, P], F32, tag="g", name=f"g_ps{b}") for b in range(2)]
            for c in range(NCHUNK):
                b = c // 2
                for tl in range(16):
                    gt = (c % 2) * 16 + tl  # accumulation index within batch
                    blk = xn_tiles[c][:, tl * P : (tl + 1) * P]
                    nc.tensor.matmul(
                        g_ps[b][:], blk, blk, start=(gt == 0), stop=(gt == 31)
                    )

            # bf16 partials -> 64KB AllReduce payload
            g_sb = work.tile([P, 256], BF, tag="gsb")
            for b in range(2):
                nc.vector.tensor_copy(g_sb[:, b * P : (b + 1) * P], g_ps[b][:])

            # ---- phase C: AllReduce the Gram over the 8 cores ----
            g_in = dram.tile([P, 256], BF, tag="gin")
            g_out = dram.tile([P, 256], BF, tag="gout", addr_space="Shared")
            g_dma = nc.sync.dma_start(g_in[:], g_sb[:])
            nc.gpsimd.collective_compute(
                "AllReduce",
                mybir.AluOpType.add,
                ins=[g_in.opt()],
                outs=[g_out.opt()],
                replica_groups=[list(range(NCORES))],
            )
            gbf = [
                work.tile([P, P], BF, tag=f"gbf{b}", name=f"gbf{b}")
                for b in range(2)
            ]
            for b in range(2):
                eng = nc.sync if b == 0 else nc.scalar
                eng.dma_start(gbf[b][:], g_out[:, b * P : (b + 1) * P])

            # ---- transpose xn -> xc in [c, n] layout, under the AR wait ----
            # Ordering-only deps on the G-path DMA keep the scheduler from
            # hoisting these ahead of the G matmuls (which would delay the
            # collective trigger); no runtime semaphore is added.
            xc = data.tile([P, F], BF, tag="xc")
            for c in range(NCHUNK):
                for tl in range(16):
                    col = c * CH + tl * P
                    tp = psy.tile([P, P], BF, tag="y", name=f"tp{c}_{tl}")
                    tri = nc.tensor.transpose(
                        tp[:], xn_tiles[c][:, tl * P : (tl + 1) * P], ident[:]
                    )
                    add_dep_helper(
                        tri.ins, g_dma.ins, sync=True,
                        reason="transposes ordered after the G path",
                    )
                    nc.vector.tensor_copy(xc[:, col : col + P], tp[:])

            # Dummy PE work to keep the HAM clock-gate warm through the
            # AllReduce wait so phases D/E run at 2.4 GHz, sized to roughly
            # the expected collective window.  fp32 matmuls run at 4
            # cycles/row (~850ns each), so few instructions cover a long
            # window.  Results are never read; the psum slots are the ones
            # the G partials released.
            last_warm = None
            for w in range(DUMMY_WARM_MMS):
                scratch = psg.tile([P, 512], F32, tag="g", name=f"warm{w}")
                wi = nc.tensor.matmul(
                    scratch[:], dummy_src[:, :P], dummy_src[:],
                    start=True, stop=True,
                )
                add_dep_helper(
                    wi.ins, g_dma.ins, sync=True,
                    reason="warm-keeper ordered after the G path",
                )
                last_warm = wi

            # ---- phase D: scores -> softmax -> W_eff (replicated, tiny) ----
            # scale folded into wqT on the host; 1/n folded into wv.
            # Batch 0/1 stages interleaved so the engines pipeline.
            sums = work.tile([P, 8], F32, tag="sums")
            recip = work.tile([P, 8], F32, tag="recip")
            weff = [
                work.tile([P, 64], BF, tag=f"weff{b}", name=f"weff{b}")
                for b in range(2)
            ]
            a_ps = [psd.tile([P, 512], F32, tag="d", name=f"a_ps{b}") for b in range(2)]
            a_sb = [work.tile([P, 512], BF, tag=f"asb{b}", name=f"a_sb{b}") for b in range(2)]
            s_ps = [psd.tile([P, 256], F32, tag="d", name=f"s_ps{b}") for b in range(2)]
            negmax = [work.tile([P, 4], F32, tag=f"nm{b}", name=f"negmax{b}") for b in range(2)]
            exp_sb = [work.tile([P, 256], F32, tag=f"exp{b}", name=f"exp_sb{b}") for b in range(2)]
            attn = [work.tile([P, 256], BF, tag=f"attn{b}", name=f"attn{b}") for b in range(2)]
            mt_ps = [psd.tile([P, 256], F32, tag="d2", name=f"mt_ps{b}") for b in range(2)]
            mt_sb = [work.tile([P, 256], BF, tag=f"mt{b}", name=f"mt_sb{b}") for b in range(2)]
            w_ps = [psd.tile([P, 64], F32, tag="d2", name=f"w_ps{b}") for b in range(2)]

            for b in range(2):
                ai = nc.tensor.matmul(
                    a_ps[b][:], gbf[b][:], wq[:],
                    start=True, stop=True,
                )
                if last_warm is not None:
                    add_dep_helper(
                        ai.ins, last_warm.ins, sync=False,
                        reason="phase D after the warm-keeper block",
                    )
            for b in range(2):
                # sliced so the first S matmuls start after slice 0 lands
                for sl in range(4):
                    nc.vector.tensor_copy(
                        a_sb[b][:, sl * 128 : (sl + 1) * 128],
                        a_ps[b][:, sl * 128 : (sl + 1) * 128],
                    )
            # S[i-half, j-group]: head h at partitions 64*(h%2), cols 64*(h//2)
            for b in range(2):
                for h in range(HEADS):
                    pb = 64 * (h % 2)
                    cg = 64 * (h // 2)
                    nc.tensor.matmul(
                        s_ps[b][pb : pb + 64, cg : cg + 64],
                        a_sb[b][:, h * 64 : (h + 1) * 64],
                        wk[:, h * 64 : (h + 1) * 64],
                        start=True, stop=True,
                    )
            # Per-group max subtracted on DVE (cheap, parallel engine) so the
            # exp is ONE wide ACT op per batch instead of 8 serialized ones.
            sm_sb = [work.tile([P, 256], F32, tag=f"sm{b}", name=f"sm_sb{b}") for b in range(2)]
            for b in range(2):
                nc.vector.reduce_max(
                    negmax[b][:],
                    s_ps[b][:].rearrange("p (g j) -> p g j", j=64),
                    axis=mybir.AxisListType.X,
                    negate=True,
                )
            for b in range(2):
                nc.vector.tensor_tensor(
                    sm_sb[b][:].rearrange("p (g j) -> p g j", j=64),
                    s_ps[b][:].rearrange("p (g j) -> p g j", j=64),
                    negmax[b][:].rearrange("p g -> p g ()").broadcast_to((P, 4, 64)),
                    op=mybir.AluOpType.add,
                )
            for b in range(2):
                nc.scalar.activation(
                    exp_sb[b][:],
                    sm_sb[b][:],
                    mybir.ActivationFunctionType.Exp,
                    bias=0.0,
                    scale=1.0,
                )
            for b in range(2):
                nc.vector.reduce_sum(
                    sums[:, b * 4 : (b + 1) * 4],
                    exp_sb[b][:].rearrange("p (g j) -> p g j", j=64),
                    axis=mybir.AxisListType.X,
                )
            for b in range(2):
                nc.vector.reciprocal(
                    recip[:, b * 4 : (b + 1) * 4], sums[:, b * 4 : (b + 1) * 4]
                )
            for b in range(2):
                nc.vector.tensor_tensor(
                    attn[b][:].rearrange("p (g j) -> p g j", j=64),
                    exp_sb[b][:].rearrange("p (g j) -> p g j", j=64),
                    recip[:, b * 4 : (b + 1) * 4]
                    .rearrange("p g -> p g ()")
                    .broadcast_to((P, 4, 64)),
                    op=mybir.AluOpType.mult,
                )
            # MT_bh = attn_bh^T @ WoutT_h, same packing as attn/woT
            for b in range(2):
                for h in range(HEADS):
                    pb = 64 * (h % 2)
                    cg = 64 * (h // 2)
                    nc.tensor.matmul(
                        mt_ps[b][pb : pb + 64, cg : cg + 64],
                        attn[b][pb : pb + 64, cg : cg + 64],
                        wo[pb : pb + 64, cg : cg + 64],
                        start=True, stop=True,
                    )
            for b in range(2):
                nc.vector.tensor_copy(mt_sb[b][:], mt_ps[b][:])
            # W_effT_b[c, o] accumulated over the 4 head-pair chunks
            for b in range(2):
                for g in range(4):
                    nc.tensor.matmul(
                        w_ps[b][:],
                        wv[:, g * P : (g + 1) * P],
                        mt_sb[b][:, g * 64 : (g + 1) * 64],
                        start=(g == 0), stop=(g == 3),
                    )
            for b in range(2):
                nc.vector.tensor_copy(weff[b][:], w_ps[b][:])

            # ---- phase E: y = W_eff @ x + b_out, chunked + streamed out ----
            for j in range(8):
                y_ps = psy.tile([P, 512], F32, tag="y", name=f"y_ps{j}")
                for b in range(2):
                    nc.tensor.matmul(
                        y_ps[b * 64 : (b + 1) * 64, :],
                        weff[b][:],
                        xc[:, b * NSH + j * 512 : b * NSH + (j + 1) * 512],
                        start=True, stop=True,
                    )
                y_sb = ypool.tile([P, 512], F32, tag=f"y{j}", name=f"y_sb{j}")
                nc.any.tensor_scalar_add(y_sb[:], y_ps[:], bo[:, 0:1])
                if j < 7:
                    eng = nc.sync if j % 2 == 0 else nc.scalar
                    eng.dma_start(out_ext[:, j * 512 : (j + 1) * 512], y_sb[:])
                else:
                    # split the final chunk across both rings to shorten the
                    # tail (its DMA is the last data movement in the kernel)
                    nc.sync.dma_start(
                        out_ext[:, j * 512 : j * 512 + 256], y_sb[:, 0:256]
                    )
                    nc.scalar.dma_start(
                        out_ext[:, j * 512 + 256 : (j + 1) * 512], y_sb[:, 256:512]
                    )

            # consume the ncfw warm-up collective's output so nothing prunes
            # it; by now its mesh has long completed, so this is free.
            warm_sink = work.tile([P, 2], F32, tag="warm_sink")
            nc.sync.dma_start(warm_sink[:], warm_out[:])

    nc.compile()
    return nc


def _get_nc():
    global _CACHED_NC
    if _CACHED_NC is None:
        _CACHED_NC = build_nc()
    return _CACHED_NC


def make_in_maps(x, w_qkv, w_out, b_out):
    x = np.ascontiguousarray(x, dtype=np.float32)
    w_qkv = np.asarray(w_qkv, dtype=np.float32)
    w_out = np.asarray(w_out, dtype=np.float32)
    b_out = np.asarray(b_out, dtype=np.float32)
    xf = x.reshape(2, P, N_TOT)

    wq_h = np.ascontiguousarray((w_qkv[:512].T * SCALE)).astype(bf16)
    wk_h = np.ascontiguousarray(w_qkv[512:1024].T).astype(bf16)
    wv_h = np.ascontiguousarray(
        (w_qkv[1024:] / N_TOT).reshape(4, P, P).transpose(1, 0, 2).reshape(P, 512)
    ).astype(bf16)
    wo_f = np.zeros((P, 256), np.float32)
    for h in range(HEADS):
        wo_f[
            64 * (h % 2) : 64 * (h % 2) + 64, 64 * (h // 2) : 64 * (h // 2) + 64
        ] = w_out[:, h * 64 : (h + 1) * 64].T
    wo_h = wo_f.astype(bf16)
    bo_h = np.concatenate([b_out, b_out]).reshape(P, 1).astype(np.float32)
    id_h = np.eye(P, dtype=np.float32).astype(bf16)

    in_maps = []
    for c in range(NCORES):
        sh = xf[:, :, c * NSH : (c + 1) * NSH]  # (2, 128, 4096)
        xn_h = np.ascontiguousarray(
            sh.transpose(0, 2, 1)
            .reshape(2, 32, P, P)
            .transpose(2, 0, 1, 3)
            .reshape(P, F)
        ).astype(bf16)
        in_maps.append(
            {
                "xn": xn_h,
                "wqT": wq_h,
                "wkT": wk_h,
                "wv": wv_h,
                "woT": wo_h,
                "bout": bo_h,
                "ident": id_h,
            }
        )
    return in_maps


def assemble_output(results):
    y = np.empty((2, 64, N_TOT), np.float32)
    for c in range(NCORES):
        o = np.asarray(results[c]["out"])  # [128, 4096]
        y[0, :, c * NSH : (c + 1) * NSH] = o[:64]
        y[1, :, c * NSH : (c + 1) * NSH] = o[64:]
    return y.reshape(2, 64, 32, 32, 32)


def kernel(**inputs):
    in_maps = make_in_maps(
        inputs["x"], inputs["w_qkv"], inputs["w_out"], inputs["b_out"]
    )
    nc = _get_nc()
    res = run_bass_kernel_spmd(nc, in_maps, core_ids=list(range(NCORES)))
    return assemble_output(res.results)

